# revision 8
# baseline (speedup 1.0000x reference)
"""BiGRU (S=512, B=64, I=256, H=512, L=2) Trainium2 Bass kernel.

Strategy: 4-way batch split x 2-way direction split across 8 NeuronCores.
Cores 0-3 run the forward GRU chain (layers 0 and 1) for batch quarters
0-3; cores 4-7 run the backward chain (fed time-reversed input, so the
device program is identical on every core).  Per layer each core does:

  P-phase: gxT = Wih @ xT + bias  (big efficient matmul, bf16, weights
           stationary, all timesteps as the moving operand)
  S-phase: 512-step sequential GRU scan.  gh.T chunks [128, B] are
           produced with Whh tiles stationary (output transposed so the
           gate elementwise math runs with the gate dim on partitions).

Between layers the forward/backward partners exchange their hidden-state
sequences with a pairwise AllGather (written in the partner's processing
order, so each side reads sequentially).  Final un-transpose / un-reverse
of the output happens on the host.
"""

import os
import sys
import numpy as np

for _p in ("/opt/trn_rl_repo", "/root/.axon_site/_ro/trn_rl_repo"):
    if os.path.isdir(_p) and _p not in sys.path:
        sys.path.insert(0, _p)

import ml_dtypes
from contextlib import ExitStack

import concourse.bass as bass
import concourse.tile as tile
from concourse import bacc, mybir
from concourse.bass import ts
from concourse.bass_utils import run_bass_kernel_spmd

BF16 = mybir.dt.bfloat16
F32 = mybir.dt.float32
AF = mybir.ActivationFunctionType
ALU = mybir.AluOpType

S, B, I, H, L = 512, 64, 256, 512, 2
G = 3 * H            # 1536 gate rows (r, z, n)
NCORE = 8
BQ = B // 4          # 16 batch per core
SB = S * BQ          # 8192 moving columns
F = H // 128         # 4 h-fold chunks
M12 = G // 128       # 12 gate chunks
KI0 = I // 128       # 2 contraction chunks, layer-0 input proj
KI1 = 2 * H // 128   # 8 contraction chunks, layer-1 input proj
NCOL = 512           # P-phase moving chunk width
TBLK = 8             # S-phase gx prefetch / y1 writeback block (steps)

# m-chunk emission order in the scan: n-gates first (their consumer chain is
# longest), then r, then z — lets gate math overlap the remaining matmuls.
SCAN_M_ORDER = [8, 9, 10, 11, 0, 1, 2, 3, 4, 5, 6, 7]


def _p_phase(ctx, tc, nc, wT_dram, gbias_dram, gx_dram, ki, rhs_fn, tag):
    """gxT[m*128+p, c] = sum_k W.T[k,:].T... : out = W @ xT + bias, bf16."""
    nc_ = nc
    wpool = ctx.enter_context(tc.tile_pool(name=f"w_{tag}", bufs=1))
    bpool = ctx.enter_context(tc.tile_pool(name=f"b_{tag}", bufs=1))
    psum = ctx.enter_context(tc.tile_pool(name=f"ps_{tag}", bufs=4, space="PSUM"))
    stg = ctx.enter_context(tc.tile_pool(name=f"st_{tag}", bufs=4))

    wsb = wpool.tile([128, ki, G], BF16)
    nc_.sync.dma_start(wsb[:], wT_dram.ap().rearrange("(k p) g -> p k g", p=128))
    gb = bpool.tile([128, M12], F32)
    nc_.sync.dma_start(gb[:], gbias_dram.ap())

    gx_r = gx_dram.ap().rearrange("(m p) c -> p m c", p=128)
    ncch = SB // NCOL
    for c in range(ncch):
        rhs_tiles = rhs_fn(c)  # list of ki APs, each [128, NCOL] bf16
        for m in range(M12):
            ps = psum.tile([128, NCOL], F32)
            for k in range(ki):
                nc_.tensor.matmul(
                    ps[:],
                    lhsT=wsb[:, k, ts(m, 128)],
                    rhs=rhs_tiles[k],
                    start=(k == 0),
                    stop=(k == ki - 1),
                )
            out = stg.tile([128, NCOL], BF16)
            if m % 2 == 0:
                nc_.scalar.activation(out[:], ps[:], AF.Identity, bias=gb[:, m : m + 1])
            else:
                nc_.vector.tensor_scalar_add(out[:], ps[:], gb[:, m : m + 1])
            nc_.sync.dma_start(gx_r[:, m, ts(c, NCOL)], out[:])


def _s_phase(ctx, tc, nc, whhT_dram, nbias_dram, gx_dram, layer, y0own, y1T_dram):
    """512-step GRU scan.  layer==0: h states written (bf16) into y0own SBUF
    tile.  layer==1: h states written (f32) to y1T_dram, bf16 state kept in
    ping-pong tiles."""
    nc_ = nc
    tag = f"s{layer}"
    wpool = ctx.enter_context(tc.tile_pool(name=f"whh_{tag}", bufs=1))
    cpool = ctx.enter_context(tc.tile_pool(name=f"c_{tag}", bufs=1))
    gxp = ctx.enter_context(tc.tile_pool(name=f"gx_{tag}", bufs=3))
    psum = ctx.enter_context(tc.tile_pool(name=f"ps_{tag}", bufs=2, space="PSUM"))
    gp = ctx.enter_context(tc.tile_pool(name=f"g_{tag}", bufs=3))
    hp = ctx.enter_context(tc.tile_pool(name=f"h_{tag}", bufs=3))
    yp = ctx.enter_context(tc.tile_pool(name=f"y_{tag}", bufs=3))

    whh = wpool.tile([128, F, G], BF16)
    nc_.sync.dma_start(whh[:], whhT_dram.ap().rearrange("(k p) g -> p k g", p=128))
    # nbias comes pre-broadcast from the host as [128, F*BQ]
    nbx = cpool.tile([128, F, BQ], F32)
    nc_.sync.dma_start(nbx[:], nbias_dram.ap().rearrange("p (f b) -> p f b", b=BQ))
    zero_bf = cpool.tile([128, F, BQ], BF16)
    nc_.vector.memset(zero_bf[:], 0.0)
    zero_f32 = cpool.tile([128, F, BQ], F32)
    nc_.vector.memset(zero_f32[:], 0.0)

    gx_r = gx_dram.ap().rearrange("(m p) c -> p m c", p=128)
    y1_r = None
    if y1T_dram is not None:
        y1_r = y1T_dram.ap().rearrange("(f p) c -> p f c", p=128)

    h_f32_prev = zero_f32
    h_bf_prev = zero_bf
    gx_t = None
    y1sb = None
    for u in range(S):
        j = u % TBLK
        if j == 0:
            gx_t = gxp.tile([128, M12, TBLK * BQ], BF16)
            nc_.sync.dma_start(gx_t[:], gx_r[:, :, ts(u // TBLK, TBLK * BQ)])
            if layer == 1:
                y1sb = yp.tile([128, F, TBLK * BQ], F32, tag="y1sb")

        if layer == 0:
            hprev = y0own[:, :, ts(u - 1, BQ)] if u > 0 else zero_bf[:]
        else:
            hprev = h_bf_prev[:]

        gh = psum.tile([128, M12, BQ], F32)
        for m in SCAN_M_ORDER:
            for k in range(F):
                nc_.tensor.matmul(
                    gh[:, m, :],
                    lhsT=whh[:, k, ts(m, 128)],
                    rhs=hprev[:, k, :],
                    start=(k == 0),
                    stop=(k == F - 1),
                )

        col = ts(j, BQ)
        # gate math, all tiles [128, F, BQ]
        hnb = gp.tile([128, F, BQ], F32, tag="hnb")
        nc_.vector.tensor_tensor(hnb[:], gh[:, 2 * F : 3 * F, :], nbx[:], ALU.add)
        t_r = gp.tile([128, F, BQ], F32, tag="t_r")
        nc_.vector.tensor_tensor(t_r[:], gh[:, 0:F, :], gx_t[:, 0:F, col], ALU.add)
        r = gp.tile([128, F, BQ], F32, tag="r")
        nc_.scalar.activation(r[:], t_r[:], AF.Sigmoid)
        tn = gp.tile([128, F, BQ], F32, tag="tn")
        nc_.vector.tensor_tensor(tn[:], r[:], hnb[:], ALU.mult)
        tn2 = gp.tile([128, F, BQ], F32, tag="tn2")
        nc_.vector.tensor_tensor(tn2[:], tn[:], gx_t[:, 2 * F : 3 * F, col], ALU.add)
        n = gp.tile([128, F, BQ], F32, tag="n")
        nc_.scalar.activation(n[:], tn2[:], AF.Tanh)
        t_z = gp.tile([128, F, BQ], F32, tag="t_z")
        nc_.vector.tensor_tensor(t_z[:], gh[:, F : 2 * F, :], gx_t[:, F : 2 * F, col], ALU.add)
        z = gp.tile([128, F, BQ], F32, tag="z")
        nc_.scalar.activation(z[:], t_z[:], AF.Sigmoid)
        d = gp.tile([128, F, BQ], F32, tag="d")
        nc_.vector.tensor_tensor(d[:], h_f32_prev[:], n[:], ALU.subtract)
        e = gp.tile([128, F, BQ], F32, tag="e")
        nc_.vector.tensor_tensor(e[:], z[:], d[:], ALU.mult)

        if layer == 0:
            hnew = hp.tile([128, F, BQ], F32, tag="hnew")
            nc_.vector.tensor_tensor(hnew[:], n[:], e[:], ALU.add)
            nc_.scalar.activation(y0own[:, :, ts(u, BQ)], hnew[:], AF.Copy)
            h_f32_prev = hnew
        else:
            hv = y1sb[:, :, col]
            nc_.vector.tensor_tensor(hv, n[:], e[:], ALU.add)
            hb = hp.tile([128, F, BQ], BF16, tag="hbf")
            nc_.scalar.activation(hb[:], hv, AF.Copy)
            h_bf_prev = hb
            h_f32_prev = hv
            if j == TBLK - 1:
                nc_.sync.dma_start(y1_r[:, :, ts(u // TBLK, TBLK * BQ)], y1sb[:])


def build_program(debug=False):
    nc = bacc.Bacc("TRN2", target_bir_lowering=False, debug=debug,
                   num_devices=NCORE)

    def din(name, shape, dt):
        return nc.dram_tensor(name, list(shape), dt, kind="ExternalInput")

    xT = din("xT", (I, SB), BF16)
    wih0T = din("wih0T", (I, G), BF16)
    whh0T = din("whh0T", (H, G), BF16)
    wih1T = din("wih1T", (2 * H, G), BF16)
    whh1T = din("whh1T", (H, G), BF16)
    gbias0 = din("gbias0", (128, M12), F32)
    gbias1 = din("gbias1", (128, M12), F32)
    nbias0 = din("nbias0", (128, F * BQ), F32)
    nbias1 = din("nbias1", (128, F * BQ), F32)

    y1T = nc.dram_tensor("y1T", [H, SB], F32, kind="ExternalOutput")

    gx0T = nc.dram_tensor("gx0T", [G, SB], BF16)
    gx1T = nc.dram_tensor("gx1T", [G, SB], BF16)
    y0ex = nc.dram_tensor("y0ex", [H, SB], BF16)
    y0g = nc.dram_tensor("y0g", [2, H, SB], BF16)
    y0loc = nc.dram_tensor("y0loc", [H, SB], BF16)

    groups = [[2 * q, 2 * q + 1] for q in range(4)]

    with tile.TileContext(nc) as tc:
        with ExitStack() as ctx:
            # ---- P0: layer-0 input projection ----
            with ExitStack() as pctx:
                xpool = pctx.enter_context(tc.tile_pool(name="xsb", bufs=1))
                xsb = xpool.tile([128, KI0, SB], BF16)
                nc.sync.dma_start(xsb[:], xT.ap().rearrange("(k p) c -> p k c", p=128))
                _p_phase(pctx, tc, nc, wih0T, gbias0, gx0T, KI0,
                         lambda c: [xsb[:, k, ts(c, NCOL)] for k in range(KI0)], "p0")

            # ---- S0: layer-0 scan; y0own holds the h sequence in SBUF ----
            y0pool = ctx.enter_context(tc.tile_pool(name="y0own", bufs=1))
            y0own = y0pool.tile([128, F, SB], BF16)
            with ExitStack() as sctx:
                _s_phase(sctx, tc, nc, whh0T, nbias0, gx0T, 0, y0own, None)

            # write own h sequence to y0ex in the partner's processing order
            # (time-reversed at BQ-block granularity), one 3-dim DMA per fold
            y0e = y0ex.ap()
            for f in range(F):
                dst = bass.AP(
                    tensor=y0e.tensor,
                    offset=f * 128 * SB + (S - 1) * BQ,
                    ap=[[SB, 128], [-BQ, S], [1, BQ]],
                )
                src = y0own[:, f, :].rearrange("p (s b) -> p s b", b=BQ)
                nc.sync.dma_start(dst, src)

            # ---- exchange: pairwise AllGather + partner-half copy ----
            nc.gpsimd.collective_compute(
                "AllGather", ALU.bypass,
                ins=[y0ex.ap()], outs=[y0g.ap()],
                replica_groups=groups,
            )
            rank = nc.gpsimd.cc_rank(groups)
            with tc.If(rank < 1) as cmp:
                for rr in range(4):
                    nc.gpsimd.dma_start(
                        y0loc.ap()[ts(rr, 128), :], y0g.ap()[1, ts(rr, 128), :])
            with cmp.Else():
                for rr in range(4):
                    nc.gpsimd.dma_start(
                        y0loc.ap()[ts(rr, 128), :], y0g.ap()[0, ts(rr, 128), :])

            # ---- P1: layer-1 input projection ----
            with ExitStack() as pctx:
                ppool = pctx.enter_context(tc.tile_pool(name="part", bufs=3))
                y0l_r = y0loc.ap().rearrange("(k p) c -> p k c", p=128)

                def rhs1(c):
                    part = ppool.tile([128, F, NCOL], BF16)
                    nc.sync.dma_start(part[:], y0l_r[:, :, ts(c, NCOL)])
                    return [y0own[:, k, ts(c, NCOL)] for k in range(F)] + \
                           [part[:, k, :] for k in range(F)]

                _p_phase(pctx, tc, nc, wih1T, gbias1, gx1T, KI1, rhs1, "p1")

            # ---- S1: layer-1 scan -> y1T ----
            with ExitStack() as sctx:
                _s_phase(sctx, tc, nc, whh1T, nbias1, gx1T, 1, None, y1T)

    nc.compile()
    return nc


_PROGRAM_CACHE = {}


def _get_program():
    if "nc" not in _PROGRAM_CACHE:
        _PROGRAM_CACHE["nc"] = build_program()
    return _PROGRAM_CACHE["nc"]


def _host_inputs(inputs):
    """Build the 8 per-core input maps from the full problem inputs."""
    bf = ml_dtypes.bfloat16
    x = np.asarray(inputs["input"], np.float32)            # (S, B, I)
    in_maps = []
    for c in range(NCORE):
        fwd = c % 2 == 0
        q = c // 2
        d = "f" if fwd else "b"
        xq = x[:, q * BQ:(q + 1) * BQ, :]
        if not fwd:
            xq = xq[::-1]
        xTv = np.ascontiguousarray(xq.transpose(2, 0, 1).reshape(I, SB))

        def wT(wname):
            return np.ascontiguousarray(np.asarray(inputs[wname], np.float32).T)

        wih0 = wT(f"Wih_{d}0")        # (I, G)
        whh0 = wT(f"Whh_{d}0")        # (H, G)
        wih1_full = wT(f"Wih_{d}1")   # (2H, G); rows = y0 features [hf | hb]
        own_sl = slice(0, H) if fwd else slice(H, 2 * H)
        par_sl = slice(H, 2 * H) if fwd else slice(0, H)
        wih1 = np.concatenate([wih1_full[own_sl], wih1_full[par_sl]], axis=0)
        whh1 = wT(f"Whh_{d}1")

        def gbias(layer):
            bih = np.asarray(inputs[f"bih_{d}{layer}"], np.float32)
            bhh = np.asarray(inputs[f"bhh_{d}{layer}"], np.float32)
            gb = np.concatenate([bih[:2 * H] + bhh[:2 * H], bih[2 * H:]])
            return np.ascontiguousarray(gb.reshape(M12, 128).T)  # [128, M12]

        def nbias(layer):
            bhh = np.asarray(inputs[f"bhh_{d}{layer}"], np.float32)
            nb = bhh[2 * H:].reshape(F, 128).T  # [128, F]
            return np.ascontiguousarray(
                np.broadcast_to(nb[:, :, None], (128, F, BQ)).reshape(128, F * BQ))

        in_maps.append({
            "xT": xTv.astype(bf),
            "wih0T": wih0.astype(bf), "whh0T": whh0.astype(bf),
            "wih1T": wih1.astype(bf), "whh1T": whh1.astype(bf),
            "gbias0": gbias(0), "gbias1": gbias(1),
            "nbias0": nbias(0), "nbias1": nbias(1),
        })
    return in_maps


def kernel(**inputs) -> np.ndarray:
    nc = _get_program()
    in_maps = _host_inputs(inputs)
    trace = bool(int(os.environ.get("BIGRU_TRACE", "0")))
    kw = {}
    if trace and os.environ.get("BIGRU_TRACE_DIR"):
        kw["tmpdir"] = os.environ["BIGRU_TRACE_DIR"]
    res = run_bass_kernel_spmd(nc, in_maps, list(range(NCORE)), trace=trace, **kw)
    if trace and res.exec_time_ns is not None:
        print(f"HW exec time: {res.exec_time_ns} ns")
        _PROGRAM_CACHE["exec_time_ns"] = res.exec_time_ns
        _PROGRAM_CACHE["profile_json"] = res.profile_json

    out = np.empty((S, B, 2 * H), np.float32)
    for c in range(NCORE):
        fwd = c % 2 == 0
        q = c // 2
        y = res.results[c]["y1T"].reshape(H, S, BQ).transpose(1, 2, 0)  # (S, BQ, H)
        if not fwd:
            y = y[::-1]
        out[:, q * BQ:(q + 1) * BQ, (0 if fwd else H):(H if fwd else 2 * H)] = y
    return out
